# Initial kernel scaffold
#
"""CenterLoss (segment_reduce) Trainium2 Bass kernel.

loss = (1/N) * sum_{i,c: gt[i,c]>0} ||features[i] - centers[c]||^2

Per core (8-way data-parallel on rows, centers replicated):
  Z = mask^T @ [features_bf16 | 1 | fsq]   accumulated in PSUM over 64
  row-tiles of 128 (8 class chunks of 125 = 8 PSUM banks), where the
  int32->bf16 mask cast happens inside the SWDGE DMA itself.
  Then on-device: t3[p] = sum_{k,f} Z[125p+...,f]*centers[...],
  colcnt and fsq-weighted columns are copied out, and the final scalar
  combine (with csq from centers) runs on the host over 8x125 partials.
"""

import numpy as np

N_TOTAL = 65536
C = 1000
F = 256
NCORES = 8
NSH = N_TOTAL // NCORES  # 8192 rows per core
P = 128                  # partition tile (rows per matmul step)
T = NSH // P             # 64 row tiles per core
CCH = 125                # class chunk (PSUM partition dim)
NCH = C // CCH           # 8 class chunks == 8 PSUM banks
F2 = F + 2               # features | ones | fsq
MB = 10                  # mask tile ring depth
XB = 8                   # featx ring depth

# features stream in progressively finer-to-coarser groups so the first
# tiles are available almost immediately (tile counts, sum = T)
FEAT_GROUPS = [1, 1, 2, 4, 8, 16, 16, 16]


def build_bass():
    import concourse.bass as bass
    import concourse.mybir as mybir
    import concourse.tile as tile
    from contextlib import ExitStack

    f32 = mybir.dt.float32
    bf16 = mybir.dt.bfloat16
    i32 = mybir.dt.int32

    nc = bass.Bass(trn_type="TRN2")
    gt = nc.dram_tensor("gt", [NSH, C], i32, kind="ExternalInput")
    feat = nc.dram_tensor("features", [NSH, F], f32, kind="ExternalInput")
    cent = nc.dram_tensor("centers", [C, F], f32, kind="ExternalInput")
    out = nc.dram_tensor("partial", [CCH, 1 + 2 * NCH], f32,
                         kind="ExternalOutput")

    gt_r = gt.rearrange("(t p) c -> t p c", p=P)
    feat_r = feat.rearrange("(t p) f -> p t f", p=P)
    # chunk k, partition p  <->  class k*CCH + p
    cent_r = cent.rearrange("(k p) f -> p k f", p=CCH)

    starts = []
    s = 0
    for g in FEAT_GROUPS:
        starts.append(s)
        s += g
    boundary = {st for st in starts if st >= XB}
    fsq_fresh = {st for st in starts if st > 0}

    with tile.TileContext(nc) as tc, ExitStack() as ctx:
        const = ctx.enter_context(tc.tile_pool(name="const", bufs=1))
        mp = ctx.enter_context(tc.tile_pool(name="mp", bufs=MB))
        xp = ctx.enter_context(tc.tile_pool(name="xp", bufs=XB))
        sp = ctx.enter_context(tc.tile_pool(name="sp", bufs=4))
        ep = ctx.enter_context(tc.tile_pool(name="ep", bufs=1))
        zp = ctx.enter_context(tc.tile_pool(name="zp", bufs=1, space="PSUM"))

        # features resident in SBUF (64 KB/partition); never recycled, so
        # none of these HWDGE DMAs carries a sync-wait. Sub-MB first groups
        # land in ~µs so tile 0 can start immediately.
        feat_full = const.tile([P, T, F], f32, name="feat_full")
        cent_t = const.tile([CCH, NCH, F], f32, name="cent_t")
        # feat/cent also go through SWDGE, interleaved just-in-time with
        # the mask emissions on the Pool queue: SP executes HWDGE DMAs
        # serially (each occupies it for the whole transfer), which made
        # feature delivery lag; the SWDGE stream pipelines everything.
        feat_due = {}
        spread = [0, 0, 1, 2]
        for gi, st in enumerate(starts):
            due = spread[gi] if gi < len(spread) else max(0, st - 4)
            feat_due.setdefault(due, []).append(gi)

        # one PSUM tensor spanning all 8 banks: chunk k accumulates in
        # z_big[:, k, 0:F2]; bank stride 512 f32 keeps each matmul output
        # inside a single bank.
        z_big = zp.tile([CCH, NCH, 512], mybir.dt.float32, name="z_big")

        featx_hist = []
        for t in range(T):
            mask_t = mp.tile([P, C], bf16, name="mask_t", tag="mask")
            nc.gpsimd.dma_start(out=mask_t, in_=gt_r[t])
            for gi in feat_due.get(t, ()):
                st, g = starts[gi], FEAT_GROUPS[gi]
                nc.gpsimd.dma_start(out=feat_full[:, st:st + g, :],
                                    in_=feat_r[:, st:st + g, :])
            if t == 36:
                nc.gpsimd.dma_start(out=cent_t, in_=cent_r)

            fresh = t in boundary
            featx = xp.tile([P, F2], bf16, name="featx",
                            tag="fxb" if fresh else "fx",
                            bufs=4 if fresh else None)
            fsq = sp.tile([P, 1], f32, name="fsq",
                          tag="fsqb" if t in fsq_fresh else "fsq",
                          bufs=len(fsq_fresh) if t in fsq_fresh else None)
            sqs = sp.tile([P, F], f32, name="sqs", tag="sq", bufs=1)
            featx_hist.append(featx)

            if fresh:
                # group-boundary cast: output slot is never-recycled (only
                # the new feat group's DMA wait), and the dummy read of the
                # previous featx chains it in DVE program order so the
                # scheduler cannot hoist it ahead and stall the DVE queue.
                nc.vector.tensor_tensor(
                    featx[:, 0:F], feat_full[:, t, :],
                    featx_hist[t - 1][:, 0:F], mybir.AluOpType.bypass)
            else:
                nc.vector.tensor_copy(out=featx[:, 0:F],
                                      in_=feat_full[:, t, :])
            # ACT squares the f32 features and row-reduces via accum_out;
            # the shared sqs scratch chains the squares in ACT order.
            nc.scalar.activation(
                out=sqs, in_=feat_full[:, t, :],
                func=mybir.ActivationFunctionType.Square,
                accum_out=fsq,
            )
            nc.vector.memset(featx[:, F:F + 1], 1.0)
            nc.vector.tensor_copy(out=featx[:, F + 1:F2], in_=fsq)

            if t == T - 8:
                # chained 1-element read of centers: DVE observes the cent
                # DMA here (bounded by the featx chain) so the epilogue
                # multiply needs only the PE wait.
                cent_obs = const.tile([1, 1], f32, name="cent_obs")
                nc.vector.tensor_tensor(
                    cent_obs, cent_t[0:1, 0, 0:1],
                    featx_hist[t - 1][0:1, 0:1], mybir.AluOpType.bypass)

            for k in range(NCH):
                nc.tensor.matmul(
                    z_big[:, k, 0:F2],
                    lhsT=mask_t[:, k * CCH:(k + 1) * CCH],
                    rhs=featx[:, :],
                    start=(t == 0),
                    stop=(t == T - 1),
                )

        # ---- epilogue: one fused pass over Z against centers ----
        w = ep.tile([CCH, NCH, F], f32, name="w")
        outb = ep.tile([CCH, 1 + 2 * NCH], f32, name="outb")
        nc.vector.tensor_mul(w, z_big[:, :, 0:F], cent_t)
        nc.vector.reduce_sum(out=outb[:, 0:1], in_=w,
                             axis=mybir.AxisListType.XY)
        nc.vector.tensor_copy(out=outb[:, 1:1 + NCH], in_=z_big[:, :, F])
        nc.vector.tensor_copy(out=outb[:, 1 + NCH:1 + 2 * NCH],
                              in_=z_big[:, :, F + 1])
        nc.sync.dma_start(out=out[:, :], in_=outb)

    _fix_sync_waits(nc)
    return nc


def _fix_sync_waits(nc):
    """This walrus build rejects instructions whose embedded sync-wait list
    exceeds the (AP-size-dependent) encoding space; DMAs take only ONE.
    Sound post-scheduling reductions:

    1. In-order engines (DVE/Activation/SP) never need waits on their own
       engine-proc semaphore — dispatch and completion are FIFO.
    2. A recycling mask DMA's PE (WAR) wait subsumes the WAW on the slot's
       previous DMA: the retired matmuls read every byte of the slot, so
       that DMA necessarily completed. Keep only the PE wait.
    3. An SP DMA's DMAHW lane-reuse wait can be dropped: lane semaphores
       count cumulatively, so downstream waiters still see the right
       totals, and concurrent in-flight DMAs touch disjoint data.
    4. The kernel-tail drain only needs the completion sems of DMAs that
       write DRAM outputs; every input DMA's completion is implied by its
       consumers, which the per-engine drains already order after.
    """
    inorder = {"DVE", "Activation", "SP"}

    out_sems = set()
    for f in nc.m.functions:
        for b in f.blocks:
            for inst in b.instructions:
                if (type(inst).__name__ == "InstDMACopy"
                        and inst.outs
                        and "partial" in str(inst.outs[0].memsetref)):
                    for u in inst.sync_info.on_update:
                        out_sems.add(u.ant_name)
    assert out_sems, "no output DMA found"

    for f in nc.m.functions:
        for b in f.blocks:
            for inst in b.instructions:
                si = inst.sync_info
                if si is None:
                    continue
                waits = list(si.on_wait)
                if len(waits) <= 1:
                    continue
                eng = inst.engine.name
                tn = type(inst).__name__
                if eng in inorder:
                    pruned = [w for w in waits
                              if not w.ant_name.startswith(eng + "_")]
                    if len(pruned) != len(waits):
                        inst.sync_info = type(si)(
                            on_wait=pruned, on_update=si.on_update)
                        waits = pruned
                        si = inst.sync_info
                if tn == "InstDrain" and len(waits) > 1:
                    keep = [w for w in waits if w.ant_name in out_sems]
                    assert keep, (
                        f"drain {inst.name}: no output-DMA wait among "
                        f"{[w.ant_name for w in waits]}")
                    inst.sync_info = type(si)(
                        on_wait=keep, on_update=si.on_update)
                elif tn == "InstDMACopy" and len(waits) > 1:
                    if eng == "Pool":
                        keep = [w for w in waits
                                if w.ant_name.startswith("PE_")]
                    else:
                        keep = [w for w in waits
                                if not w.ant_name.startswith("DMAHW")]
                    assert len(keep) == 1, (
                        f"multi-wait DMA {inst.name} ({eng}) has waits "
                        f"{[w.ant_name for w in waits]}")
                    inst.sync_info = type(si)(
                        on_wait=keep, on_update=si.on_update)


def _shard_inputs(inputs):
    gt = np.ascontiguousarray(np.asarray(inputs["gt"], dtype=np.int32))
    features = np.ascontiguousarray(np.asarray(inputs["features"], dtype=np.float32))
    centers = np.ascontiguousarray(np.asarray(inputs["centers"], dtype=np.float32))
    in_maps = []
    for c in range(NCORES):
        sl = slice(c * NSH, (c + 1) * NSH)
        in_maps.append({
            "gt": gt[sl],
            "features": features[sl],
            "centers": centers,
        })
    return in_maps


def _combine(results, centers):
    """Host-side scalar combine (the all-reduce of the sharding hint).

    Per-core output [125, 17]: col 0 = t3[p] = sum_{k,f} Z[c,f]*centers[c,f]
    (c = k*125+p), cols 1:9 = colcnt[p,k], cols 9:17 = fsqsum[p,k].
    """
    csq = (centers.astype(np.float64) ** 2).sum(axis=1)  # [C]
    csq_pk = csq.reshape(NCH, CCH).T                     # [125, 8]
    t1 = t2 = t3 = 0.0
    for r in results:
        part = np.asarray(r["partial"], dtype=np.float64)
        t3 += part[:, 0].sum()
        t2 += (part[:, 1:1 + NCH] * csq_pk).sum()
        t1 += part[:, 1 + NCH:1 + 2 * NCH].sum()
    return (t1 + t2 - 2.0 * t3) / N_TOTAL


def run_spmd(inputs, trace=False):
    """Compile + run on all 8 cores. Returns (loss_scalar, BassKernelResults)."""
    from concourse.bass_utils import run_bass_kernel_spmd

    nc = build_bass()
    in_maps = _shard_inputs(inputs)
    res = run_bass_kernel_spmd(
        nc, in_maps, core_ids=list(range(NCORES)), trace=trace,
    )
    loss = _combine(res.results, np.asarray(inputs["centers"], dtype=np.float32))
    return np.array(np.float32(loss), dtype=np.float32), res


def kernel(**inputs):
    loss, _ = run_spmd(inputs, trace=False)
    return loss


if __name__ == "__main__":
    # quick CoreSim numerical check on core 0's shard
    from concourse.bass_interp import CoreSim

    rng = np.random.default_rng(0)
    gt = (rng.integers(0, 2, size=(NSH, C))).astype(np.int32)
    features = rng.standard_normal((NSH, F)).astype(np.float32)
    centers = rng.standard_normal((C, F)).astype(np.float32)

    nc = build_bass()
    sim = CoreSim(nc, require_finite=True, require_nnan=True)
    sim.tensor("gt")[:] = gt
    sim.tensor("features")[:] = features
    sim.tensor("centers")[:] = centers
    sim.simulate()

    class _R:
        results = [{"partial": np.asarray(sim.tensor("partial"))}]

    got = _combine(_R.results, centers) * N_TOTAL

    mask = (gt > 0).astype(np.float64)
    f64, c64 = features.astype(np.float64), centers.astype(np.float64)
    dist = (
        (f64 * f64).sum(1)[:, None]
        + (c64 * c64).sum(1)[None, :]
        - 2.0 * (f64 @ c64.T)
    )
    want = float((mask * dist).sum())
    print(f"sim partial sum = {got:.6e}  want = {want:.6e}  rel = {abs(got - want) / abs(want):.3e}")



# revision 1
# speedup vs baseline: 1.0609x; 1.0609x over previous
"""CenterLoss (segment_reduce) Trainium2 Bass kernel.

loss = (1/N) * sum_{i,c: gt[i,c]>0} ||features[i] - centers[c]||^2

Per core (8-way data-parallel on rows, centers replicated):
  Z = mask^T @ [features_bf16 | 1 | fsq]   accumulated in PSUM over 64
  row-tiles of 128 (8 class chunks of 125 = 8 PSUM banks), where the
  int32->bf16 mask cast happens inside the SWDGE DMA itself.
  Then on-device: t3[p] = sum_{k,f} Z[125p+...,f]*centers[...],
  colcnt and fsq-weighted columns are copied out, and the final scalar
  combine (with csq from centers) runs on the host over 8x125 partials.
"""

import numpy as np

N_TOTAL = 65536
C = 1000
F = 256
NCORES = 8
NSH = N_TOTAL // NCORES  # 8192 rows per core
P = 128                  # partition tile (rows per matmul step)
T = NSH // P             # 64 row tiles per core
CCH = 125                # class chunk (PSUM partition dim)
NCH = C // CCH           # 8 class chunks == 8 PSUM banks
F2 = F + 2               # features | ones | fsq
MB = 10                  # mask tile ring depth
XB = 8                   # featx ring depth

# features stream in progressively finer-to-coarser groups so the first
# tiles are available almost immediately (tile counts, sum = T)
FEAT_GROUPS = [1, 1, 2, 4, 8, 16, 16, 16]


def build_bass():
    import concourse.bass as bass
    import concourse.mybir as mybir
    import concourse.tile as tile
    from contextlib import ExitStack

    f32 = mybir.dt.float32
    bf16 = mybir.dt.bfloat16
    i32 = mybir.dt.int32

    nc = bass.Bass(trn_type="TRN2")
    gt = nc.dram_tensor("gt", [NSH, C], i32, kind="ExternalInput")
    feat = nc.dram_tensor("features", [NSH, F], f32, kind="ExternalInput")
    cent = nc.dram_tensor("centers", [C, F], f32, kind="ExternalInput")
    out = nc.dram_tensor("partial", [CCH, 1 + 2 * NCH], f32,
                         kind="ExternalOutput")

    gt_r = gt.rearrange("(t p) c -> t p c", p=P)
    feat_r = feat.rearrange("(t p) f -> p t f", p=P)
    # chunk k, partition p  <->  class k*CCH + p
    cent_r = cent.rearrange("(k p) f -> p k f", p=CCH)

    starts = []
    s = 0
    for g in FEAT_GROUPS:
        starts.append(s)
        s += g
    boundary = {st for st in starts if st >= XB}
    fsq_fresh = {st for st in starts if st > 0}

    with tile.TileContext(nc) as tc, ExitStack() as ctx:
        const = ctx.enter_context(tc.tile_pool(name="const", bufs=1))
        mp = ctx.enter_context(tc.tile_pool(name="mp", bufs=MB))
        xp = ctx.enter_context(tc.tile_pool(name="xp", bufs=XB))
        sp = ctx.enter_context(tc.tile_pool(name="sp", bufs=4))
        ep = ctx.enter_context(tc.tile_pool(name="ep", bufs=1))
        zp = ctx.enter_context(tc.tile_pool(name="zp", bufs=1, space="PSUM"))

        # features resident in SBUF (64 KB/partition); never recycled, so
        # none of these HWDGE DMAs carries a sync-wait. Sub-MB first groups
        # land in ~µs so tile 0 can start immediately.
        feat_full = const.tile([P, T, F], f32, name="feat_full")
        cent_t = const.tile([CCH, NCH, F], f32, name="cent_t")
        # feat/cent also go through SWDGE, interleaved just-in-time with
        # the mask emissions on the Pool queue: SP executes HWDGE DMAs
        # serially (each occupies it for the whole transfer), which made
        # feature delivery lag; the SWDGE stream pipelines everything.
        feat_due = {}
        spread = [0, 0, 1, 2]
        for gi, st in enumerate(starts):
            due = spread[gi] if gi < len(spread) else max(0, st - 4)
            feat_due.setdefault(due, []).append(gi)

        # one PSUM tensor spanning all 8 banks: chunk k accumulates in
        # z_big[:, k, 0:F2]; bank stride 512 f32 keeps each matmul output
        # inside a single bank.
        z_big = zp.tile([CCH, NCH, 512], mybir.dt.float32, name="z_big")

        featx_hist = []
        for t in range(T):
            mask_t = mp.tile([P, C], bf16, name="mask_t", tag="mask")
            nc.gpsimd.dma_start(out=mask_t, in_=gt_r[t])
            for gi in feat_due.get(t, ()):
                st, g = starts[gi], FEAT_GROUPS[gi]
                nc.gpsimd.dma_start(out=feat_full[:, st:st + g, :],
                                    in_=feat_r[:, st:st + g, :])
            if t == 36:
                nc.gpsimd.dma_start(out=cent_t, in_=cent_r)

            fresh = t in boundary
            featx = xp.tile([P, F2], bf16, name="featx",
                            tag="fxb" if fresh else "fx",
                            bufs=4 if fresh else None)
            fsq = sp.tile([P, 1], f32, name="fsq",
                          tag="fsqb" if t in fsq_fresh else "fsq",
                          bufs=len(fsq_fresh) if t in fsq_fresh else None)
            sqs = sp.tile([P, F], f32, name="sqs", tag="sq", bufs=1)
            featx_hist.append(featx)

            if fresh:
                # group-boundary cast: output slot is never-recycled (only
                # the new feat group's DMA wait), and the dummy read of the
                # previous featx chains it in DVE program order so the
                # scheduler cannot hoist it ahead and stall the DVE queue.
                nc.vector.tensor_tensor(
                    featx[:, 0:F], feat_full[:, t, :],
                    featx_hist[t - 1][:, 0:F], mybir.AluOpType.bypass)
            else:
                nc.vector.tensor_copy(out=featx[:, 0:F],
                                      in_=feat_full[:, t, :])
            # ACT squares the f32 features and row-reduces via accum_out;
            # the shared sqs scratch chains the squares in ACT order.
            nc.scalar.activation(
                out=sqs, in_=feat_full[:, t, :],
                func=mybir.ActivationFunctionType.Square,
                accum_out=fsq,
            )
            nc.vector.memset(featx[:, F:F + 1], 1.0)
            nc.vector.tensor_copy(out=featx[:, F + 1:F2], in_=fsq)

            if t == T - 8:
                # chained 1-element read of centers: DVE observes the cent
                # DMA here (bounded by the featx chain) so the epilogue
                # multiply needs only the PE wait.
                cent_obs = const.tile([1, 1], f32, name="cent_obs")
                nc.vector.tensor_tensor(
                    cent_obs, cent_t[0:1, 0, 0:1],
                    featx_hist[t - 1][0:1, 0:1], mybir.AluOpType.bypass)

            for k in range(NCH):
                nc.tensor.matmul(
                    z_big[:, k, 0:F2],
                    lhsT=mask_t[:, k * CCH:(k + 1) * CCH],
                    rhs=featx[:, :],
                    start=(t == 0),
                    stop=(t == T - 1),
                )

        # ---- epilogue: one fused pass over Z against centers ----
        w = ep.tile([CCH, NCH, F], f32, name="w")
        outb = ep.tile([CCH, 1 + 2 * NCH], f32, name="outb")
        nc.vector.tensor_mul(w, z_big[:, :, 0:F], cent_t)
        nc.vector.reduce_sum(out=outb[:, 0:1], in_=w,
                             axis=mybir.AxisListType.XY)
        nc.vector.tensor_copy(out=outb[:, 1:1 + NCH], in_=z_big[:, :, F])
        nc.vector.tensor_copy(out=outb[:, 1 + NCH:1 + 2 * NCH],
                              in_=z_big[:, :, F + 1])
        nc.sync.dma_start(out=out[:, :], in_=outb)

    _fix_sync_waits(nc)
    return nc


def _fix_sync_waits(nc):
    """This walrus build rejects instructions whose embedded sync-wait list
    exceeds the (AP-size-dependent) encoding space; DMAs take only ONE.
    Sound post-scheduling reductions:

    1. In-order engines (DVE/Activation/SP) never need waits on their own
       engine-proc semaphore — dispatch and completion are FIFO.
    2. A recycling mask DMA's PE (WAR) wait subsumes the WAW on the slot's
       previous DMA: the retired matmuls read every byte of the slot, so
       that DMA necessarily completed. Keep only the PE wait.
    3. An SP DMA's DMAHW lane-reuse wait can be dropped: lane semaphores
       count cumulatively, so downstream waiters still see the right
       totals, and concurrent in-flight DMAs touch disjoint data.
    4. The kernel-tail drain only needs the completion sems of DMAs that
       write DRAM outputs; every input DMA's completion is implied by its
       consumers, which the per-engine drains already order after.
    """
    inorder = {"DVE", "Activation", "SP"}

    out_sems = set()
    for f in nc.m.functions:
        for b in f.blocks:
            for inst in b.instructions:
                if (type(inst).__name__ == "InstDMACopy"
                        and inst.outs
                        and "partial" in str(inst.outs[0].memsetref)):
                    for u in inst.sync_info.on_update:
                        out_sems.add(u.ant_name)
    assert out_sems, "no output DMA found"

    for f in nc.m.functions:
        for b in f.blocks:
            for inst in b.instructions:
                si = inst.sync_info
                if si is None:
                    continue
                waits = list(si.on_wait)
                if len(waits) <= 1:
                    continue
                eng = inst.engine.name
                tn = type(inst).__name__
                if eng in inorder:
                    pruned = [w for w in waits
                              if not w.ant_name.startswith(eng + "_")]
                    if len(pruned) != len(waits):
                        inst.sync_info = type(si)(
                            on_wait=pruned, on_update=si.on_update)
                        waits = pruned
                        si = inst.sync_info
                if tn == "InstDrain" and len(waits) > 1:
                    keep = [w for w in waits if w.ant_name in out_sems]
                    assert keep, (
                        f"drain {inst.name}: no output-DMA wait among "
                        f"{[w.ant_name for w in waits]}")
                    inst.sync_info = type(si)(
                        on_wait=keep, on_update=si.on_update)
                elif tn == "InstDMACopy" and len(waits) > 1:
                    if eng == "Pool":
                        keep = [w for w in waits
                                if w.ant_name.startswith("PE_")]
                    else:
                        keep = [w for w in waits
                                if not w.ant_name.startswith("DMAHW")]
                    assert len(keep) == 1, (
                        f"multi-wait DMA {inst.name} ({eng}) has waits "
                        f"{[w.ant_name for w in waits]}")
                    inst.sync_info = type(si)(
                        on_wait=keep, on_update=si.on_update)


def _shard_inputs(inputs):
    gt = np.ascontiguousarray(np.asarray(inputs["gt"], dtype=np.int32))
    features = np.ascontiguousarray(np.asarray(inputs["features"], dtype=np.float32))
    centers = np.ascontiguousarray(np.asarray(inputs["centers"], dtype=np.float32))
    in_maps = []
    for c in range(NCORES):
        sl = slice(c * NSH, (c + 1) * NSH)
        in_maps.append({
            "gt": gt[sl],
            "features": features[sl],
            "centers": centers,
        })
    return in_maps


def _combine(results, centers):
    """Host-side scalar combine (the all-reduce of the sharding hint).

    Per-core output [125, 17]: col 0 = t3[p] = sum_{k,f} Z[c,f]*centers[c,f]
    (c = k*125+p), cols 1:9 = colcnt[p,k], cols 9:17 = fsqsum[p,k].
    """
    csq = (centers.astype(np.float64) ** 2).sum(axis=1)  # [C]
    csq_pk = csq.reshape(NCH, CCH).T                     # [125, 8]
    t1 = t2 = t3 = 0.0
    for r in results:
        part = np.asarray(r["partial"], dtype=np.float64)
        t3 += part[:, 0].sum()
        t2 += (part[:, 1:1 + NCH] * csq_pk).sum()
        t1 += part[:, 1 + NCH:1 + 2 * NCH].sum()
    return (t1 + t2 - 2.0 * t3) / N_TOTAL


def run_spmd(inputs, trace=False):
    """Compile + run on all 8 cores. Returns (loss_scalar, BassKernelResults)."""
    from concourse.bass_utils import run_bass_kernel_spmd

    nc = build_bass()
    in_maps = _shard_inputs(inputs)
    res = run_bass_kernel_spmd(
        nc, in_maps, core_ids=list(range(NCORES)), trace=trace,
    )
    loss = _combine(res.results, np.asarray(inputs["centers"], dtype=np.float32))
    return np.array(np.float32(loss), dtype=np.float32), res


def kernel(**inputs):
    loss, _ = run_spmd(inputs, trace=False)
    return loss


if __name__ == "__main__":
    # quick CoreSim numerical check on core 0's shard
    from concourse.bass_interp import CoreSim

    rng = np.random.default_rng(0)
    gt = (rng.integers(0, 2, size=(NSH, C))).astype(np.int32)
    features = rng.standard_normal((NSH, F)).astype(np.float32)
    centers = rng.standard_normal((C, F)).astype(np.float32)

    nc = build_bass()
    sim = CoreSim(nc, require_finite=True, require_nnan=True)
    sim.tensor("gt")[:] = gt
    sim.tensor("features")[:] = features
    sim.tensor("centers")[:] = centers
    sim.simulate()

    class _R:
        results = [{"partial": np.asarray(sim.tensor("partial"))}]

    got = _combine(_R.results, centers) * N_TOTAL

    mask = (gt > 0).astype(np.float64)
    f64, c64 = features.astype(np.float64), centers.astype(np.float64)
    dist = (
        (f64 * f64).sum(1)[:, None]
        + (c64 * c64).sum(1)[None, :]
        - 2.0 * (f64 @ c64.T)
    )
    want = float((mask * dist).sum())
    print(f"sim partial sum = {got:.6e}  want = {want:.6e}  rel = {abs(got - want) / abs(want):.3e}")

